# revision 12
# baseline (speedup 1.0000x reference)
"""Bahdanau attention on 8 Trainium2 cores (Bass/Tile), data-parallel over B.

reference (per batch b, all shapes full):
    hp  = hidden[0] @ W_h.T + b_h                    # (B, H)
    ep  = einsum('tbh,gh->btg', enc, W_e) + b_e      # (B, T, H)
    en  = tanh(hp[:, None, :] + ep)                  # (B, T, H)
    sc  = en @ v                                     # (B, T)
    out = softmax(sc, -1)[:, None, :]                # (B, 1, T)

Sharding: B=32 split 4-per-core across 8 cores; weights replicated.

v3 layout: host stages enc as bf16 (halves HBM traffic) and ships the
weights pre-transposed (W^T, layout-only) in bf16.  Each 1024-token
supergroup of one batch arrives in SBUF already transposed to
[h=128, hc, tok] via XBAR dma_start_transpose straight from DRAM, so
the PE runs nothing but ep/score matmuls (bf16, fp32 PSUM).  The two
512-token halves of a supergroup share each stationary weight load
back-to-back and accumulate into the two banks of one [128,2,512] PSUM
tile; ACT then applies tanh over both halves in one instruction (same
per-partition bias hp^T[:, b] + b_h + b_e) writing bf16 energy; [128xBC]
bf16 matmuls against v4 (v replicated in columns 0/32/64/96, zeros
elsewhere) reduce over g so batch b's scores land directly on PSUM
partition 32*b, from where a partition-aligned DVE copy parks them in
the scores tile (no SBUF-to-SBUF DMA, which would serialize against
the XBAR transposes).  Each batch's softmax is split into
chunked ACT exp instructions interleaved into the next supergroup's
tanh stream so the single-lane exp never blocks the ACT FIFO.
"""

import sys
from contextlib import ExitStack

import numpy as np

try:
    import concourse  # noqa: F401
except ImportError:  # pragma: no cover
    sys.path.insert(0, "/opt/trn_rl_repo")

import ml_dtypes

import concourse.tile as tile
from concourse import bacc, mybir
from concourse.bass import ts
from concourse.bass_utils import run_bass_kernel_spmd

H = 1024
T = 2048
B = 32
NCORES = 8
BC = B // NCORES          # batches per core
HC = H // 128             # h chunks
GC = H // 128             # g chunks
TOK = 512                 # tokens per matmul / PSUM bank
SG = 2 * TOK              # tokens per supergroup (one batch each)
NSG_PER_B = T // SG
NSG = BC * NSG_PER_B
NEXPC = 4                 # softmax exp chunks per batch

F32 = mybir.dt.float32
BF16 = mybir.dt.bfloat16
AF = mybir.ActivationFunctionType
AX = mybir.AxisListType


def build_kernel_nc(reps=1):
    nc = bacc.Bacc(
        "TRN2",
        target_bir_lowering=False,
        debug=False,
        enable_asserts=False,
        num_devices=NCORES,
    )
    enc = nc.dram_tensor("enc", [T, BC, H], BF16, kind="ExternalInput").ap()
    weT = nc.dram_tensor("WeT", [H, H], BF16, kind="ExternalInput").ap()
    whT = nc.dram_tensor("WhT", [H, H], BF16, kind="ExternalInput").ap()
    hidT = nc.dram_tensor("hidT", [H, BC], BF16, kind="ExternalInput").ap()
    bsum = nc.dram_tensor("bsum", [H], F32, kind="ExternalInput").ap()
    v4d = nc.dram_tensor("v4", [128, GC * 128], BF16, kind="ExternalInput").ap()
    out = nc.dram_tensor("out", [BC, T], F32, kind="ExternalOutput").ap()

    with tile.TileContext(nc) as tc:
        _kernel_body(tc, enc, weT, whT, hidT, bsum, v4d, out, reps=reps)
    nc.compile()
    return nc


def _kernel_body(tc, enc, weT, whT, hidT, bsum, v4d, out, reps=1):
    nc = tc.nc
    with ExitStack() as ctx:
        singles = ctx.enter_context(tc.tile_pool(name="singles", bufs=1))

        # ---- persistent SBUF tensors -------------------------------------
        WeT = singles.tile([128, HC, H], BF16)     # WeT[h, hc, g] = W_e[g, 128*hc+h]
        # v4[p, gc, 32*b] = v[gc*128+p] for every b (else 0): the score
        # matmul against energy then lands batch data on PSUM partitions
        # 0/32/64/96 simultaneously; we only read row 32*b for batch b.
        v4 = singles.tile([128, GC, 128], BF16)
        bias_all = singles.tile([128, GC, BC], F32)  # hp^T + b_h + b_e
        bsum_sb = singles.tile([128, GC], F32)     # (b_h + b_e) chunked
        # batch b's scores live on partition 32*b so per-batch softmax can
        # run as soon as that batch's groups finish (engine ops only accept
        # partition bases 0/32/64/96; DMA scatters the rows there)
        scores = singles.tile([128, T], F32)
        probs = singles.tile([128, T], F32)
        negmax = singles.tile([128, 1], F32)
        sums = singles.tile([128, NEXPC], F32)
        stot = singles.tile([128, 1], F32)
        rsum = singles.tile([128, 1], F32)

        # ---- stage 0: weight loads + hp + bias ---------------------------
        nc.sync.dma_start(
            out=WeT[:], in_=weT.rearrange("(hc p) g -> p hc g", p=128)
        )
        nc.sync.dma_start(
            out=v4[:], in_=v4d.rearrange("p (gc c) -> p gc c", c=128)
        )
        nc.sync.dma_start(out=bsum_sb[:], in_=bsum.rearrange("(c p) -> p c", p=128))

        with tc.tile_pool(name="stage0", bufs=1) as wload, tc.tile_pool(
            name="hpps", bufs=2, space="PSUM"
        ) as hp_pool:
            WhT = wload.tile([128, HC, H], BF16)
            hidT_sb = wload.tile([128, HC, BC], BF16)
            nc.sync.dma_start(
                out=WhT[:], in_=whT.rearrange("(hc p) g -> p hc g", p=128)
            )
            nc.sync.dma_start(
                out=hidT_sb[:], in_=hidT.rearrange("(hc p) b -> p hc b", p=128)
            )
            # hp^T[g, b] accumulated over h chunks
            for gc in range(GC):
                hp_ps = hp_pool.tile([128, BC], F32)
                for hc in range(HC):
                    nc.tensor.matmul(
                        hp_ps[:],
                        WhT[:, hc, ts(gc, 128)],
                        hidT_sb[:, hc, :],
                        start=(hc == 0),
                        stop=(hc == HC - 1),
                    )
                nc.vector.tensor_scalar(
                    out=bias_all[:, gc, :],
                    in0=hp_ps[:],
                    scalar1=bsum_sb[:, gc : gc + 1],
                    scalar2=None,
                    op0=mybir.AluOpType.add,
                )

        # ---- main loop: 8 supergroups of 1024 tokens ---------------------
        # Iteration s emits: transpose-DMA(s+2), then per gc the 16 ep
        # matmuls (two 512-halves, shared stationary), the two deferred
        # sc matmuls of gc-1, one fused tanh, and (when a batch just
        # finished) one interleaved softmax-exp chunk.
        encT_pool = ctx.enter_context(tc.tile_pool(name="encT", bufs=3))
        energy_pool = ctx.enter_context(tc.tile_pool(name="energy", bufs=3))
        ep_pool = ctx.enter_context(tc.tile_pool(name="epps", bufs=2, space="PSUM"))
        sc_pool = ctx.enter_context(tc.tile_pool(name="scps", bufs=2, space="PSUM"))

        n_total = reps * NSG

        def issue_load(sg):
            s = sg % NSG
            b = s // NSG_PER_B
            t0 = (s % NSG_PER_B) * SG
            encT = encT_pool.tile([128, HC, 2, TOK], BF16, tag="encT")
            for half in range(2):
                th = t0 + half * TOK
                nc.sync.dma_start_transpose(
                    out=encT[:, :, half, :], in_=enc[th : th + TOK, b, :]
                )
            return encT

        def softmax_head(b):
            # negmax must precede the interleaved exp chunks
            r = slice(32 * b, 32 * b + 1)
            nc.vector.tensor_reduce(
                out=negmax[r], in_=scores[r, :], axis=AX.X,
                op=mybir.AluOpType.max, negate=True,
            )

        def softmax_chunk(b, c):
            r = slice(32 * b, 32 * b + 1)
            w = T // NEXPC
            nc.scalar.activation(
                out=probs[r, ts(c, w)], in_=scores[r, ts(c, w)], func=AF.Exp,
                bias=negmax[r], scale=1.0, accum_out=sums[r, c : c + 1],
            )

        def softmax_tail(b):
            r = slice(32 * b, 32 * b + 1)
            nc.vector.tensor_reduce(
                out=stot[r], in_=sums[r, :], axis=AX.X, op=mybir.AluOpType.add,
            )
            nc.vector.reciprocal(out=rsum[r], in_=stot[r])
            nc.vector.tensor_scalar_mul(probs[r, :], probs[r, :], rsum[r])
            nc.sync.dma_start(out=out[b : b + 1, :], in_=probs[r, :])

        def flush_carry(c):
            c_sc_ps, c_gc, c_energy, c_b, c_t0 = c
            for half in range(2):
                nc.tensor.matmul(
                    c_sc_ps[:, half, :], v4[:, c_gc, :],
                    c_energy[:, half, :], start=False, stop=True,
                )
            r = slice(32 * c_b, 32 * c_b + 1)
            nc.vector.tensor_copy(
                scores[r, c_t0 : c_t0 + SG], c_sc_ps[r, :, :]
            )
            return c_b if c_t0 == T - SG else None

        encTs = [issue_load(0), issue_load(1)]
        carry = None          # deferred final sc-mms of the previous group
        sm_batch = None       # batch whose softmax-exp chunks are pending

        for sg in range(n_total):
            s = sg % NSG
            b = s // NSG_PER_B
            t0 = (s % NSG_PER_B) * SG

            if sg + 2 < n_total:
                encTs.append(issue_load(sg + 2))
            encT_cur = encTs[sg]
            finished = None
            if carry is not None:
                finished = flush_carry(carry)
                carry = None
            if finished is not None:
                softmax_head(finished)
                sm_batch = finished

            sc_ps = sc_pool.tile([128, 2, TOK], F32)
            pending = None
            for gc in range(GC):
                ep_ps = ep_pool.tile([128, 2, TOK], F32)
                for hc in range(HC):
                    for half in range(2):
                        nc.tensor.matmul(
                            ep_ps[:, half, :],
                            WeT[:, hc, ts(gc, 128)],
                            encT_cur[:, hc, half, :],
                            start=(hc == 0),
                            stop=(hc == HC - 1),
                        )
                if pending is not None:
                    pc, penergy = pending
                    for half in range(2):
                        nc.tensor.matmul(
                            sc_ps[:, half, :], v4[:, pc, :],
                            penergy[:, half, :], start=(pc == 0), stop=False,
                        )
                energy = energy_pool.tile([128, 2, TOK], BF16)
                nc.scalar.activation(
                    out=energy[:],
                    in_=ep_ps[:],
                    func=AF.Tanh,
                    bias=bias_all[:, gc, b : b + 1],
                    scale=1.0,
                )
                if sm_batch is not None and gc % 2 == 1:
                    softmax_chunk(sm_batch, gc // 2)
                    if gc == GC - 1:
                        softmax_tail(sm_batch)
                        sm_batch = None
                pending = (gc, energy)
            pc, penergy = pending
            carry = (sc_ps, pc, penergy, b, t0)
            encTs[sg] = None  # release reference

        finished = flush_carry(carry)
        if finished is not None:
            softmax_head(finished)
            for c in range(NEXPC):
                softmax_chunk(finished, c)
            softmax_tail(finished)


_NC_CACHE = None


def _get_nc():
    global _NC_CACHE
    if _NC_CACHE is None:
        _NC_CACHE = build_kernel_nc()
    return _NC_CACHE


def make_in_maps(hidden, encoder_outputs, W_h, b_h, W_e, b_e, v):
    hidden = np.asarray(hidden, dtype=np.float32)
    enc = np.asarray(encoder_outputs, dtype=np.float32)
    W_h = np.asarray(W_h, dtype=np.float32)
    W_e = np.asarray(W_e, dtype=np.float32)
    b_h = np.asarray(b_h, dtype=np.float32)
    b_e = np.asarray(b_e, dtype=np.float32)
    v = np.asarray(v, dtype=np.float32)

    enc_bf = enc.astype(ml_dtypes.bfloat16)
    weT = np.ascontiguousarray(W_e.T).astype(ml_dtypes.bfloat16)
    whT = np.ascontiguousarray(W_h.T).astype(ml_dtypes.bfloat16)
    bsum = np.ascontiguousarray(b_h + b_e)
    hid0 = hidden.reshape(B, H)

    # v4[p, gc, 32*b] = v[gc*128 + p] for every b (else 0)
    v4 = np.zeros((128, GC, 128), dtype=np.float32)
    vc = v.reshape(GC, 128)  # [gc, p]
    for b in range(BC):
        v4[:, :, 32 * b] = vc.T
    v4 = np.ascontiguousarray(v4.reshape(128, GC * 128)).astype(
        ml_dtypes.bfloat16
    )

    in_maps = []
    for c in range(NCORES):
        hidT = np.ascontiguousarray(
            hid0[c * BC : (c + 1) * BC, :].T
        ).astype(ml_dtypes.bfloat16)
        in_maps.append(
            {
                "enc": np.ascontiguousarray(enc_bf[:, c * BC : (c + 1) * BC, :]),
                "WeT": weT,
                "WhT": whT,
                "hidT": hidT,
                "bsum": bsum,
                "v4": v4,
            }
        )
    return in_maps


def kernel(hidden, encoder_outputs, W_h, b_h, W_e, b_e, v):
    nc = _get_nc()
    in_maps = make_in_maps(hidden, encoder_outputs, W_h, b_h, W_e, b_e, v)
    res = run_bass_kernel_spmd(nc, in_maps, list(range(NCORES)))
    full = np.concatenate([res.results[c]["out"] for c in range(NCORES)], axis=0)
    return full[:, None, :].astype(np.float32)


# revision 20
# speedup vs baseline: 8.4832x; 8.4832x over previous
"""Bahdanau attention on 8 Trainium2 cores (Bass/Tile), data-parallel over B.

reference (per batch b, all shapes full):
    hp  = hidden[0] @ W_h.T + b_h                    # (B, H)
    ep  = einsum('tbh,gh->btg', enc, W_e) + b_e      # (B, T, H)
    en  = tanh(hp[:, None, :] + ep)                  # (B, T, H)
    sc  = en @ v                                     # (B, T)
    out = softmax(sc, -1)[:, None, :]                # (B, 1, T)

Sharding: B=32 split 4-per-core across 8 cores; weights replicated.

v7 layout: host stages enc as bf16, weights pre-transposed bf16, and v
pre-replicated to all 128 partitions (all layout-only).  Each
1024-token supergroup arrives in SBUF transposed to [h=128, hc, tok]
via XBAR dma_start_transpose.  ep is computed TRANSPOSED, [tok=128, g]:
per (tblock, ghalf) chain the PE runs 8 accumulating bf16 matmuls with
the encT block stationary and WeT moving - the PE executes nothing but
the irreducible GEMM rows (128 matmuls/supergroup, no score matmuls).
DVE then adds the per-g bias row (hp[b]+b_h+b_e, replicated across
partitions in stage 0 by an all-ones [128x128] matmul over a
zero-padded row bounced through DRAM scratch), ACT applies tanh, DVE
multiplies by v_rep, and a DVE free-axis tensor_reduce over the two
512-wide halves lands the score column of scores2[p, b, c] (token
t = c*128 + p).  When a batch finishes, its scores2 bounce through
DRAM scratch into the [1, 2048] row at partition 32*b and the
chunked-exp softmax (negmax on DVE, four ACT exp chunks interleaved
into the next supergroup's tanh stream, DVE normalize) produces the
output row.
"""

import sys
from contextlib import ExitStack

import numpy as np

try:
    import concourse  # noqa: F401
except ImportError:  # pragma: no cover
    sys.path.insert(0, "/opt/trn_rl_repo")

import ml_dtypes

import concourse.tile as tile
from concourse import bacc, mybir
from concourse.bass import ts
from concourse.bass_utils import run_bass_kernel_spmd

H = 1024
T = 2048
B = 32
NCORES = 8
BC = B // NCORES          # batches per core
HC = H // 128             # h chunks
GC = H // 128             # g chunks
TOK = 512                 # g columns per chain / PSUM bank
SG = 1024                 # tokens per supergroup (one batch each)
NSG_PER_B = T // SG
NSG = BC * NSG_PER_B
TB = SG // 128            # 128-token stationary blocks per supergroup
GH = H // TOK             # 512-wide g halves
CPB = T // 128            # score columns per batch (t = c*128 + p)
NEXPC = 4                 # softmax exp chunks per batch

F32 = mybir.dt.float32
BF16 = mybir.dt.bfloat16
AF = mybir.ActivationFunctionType
AX = mybir.AxisListType
ADD = mybir.AluOpType.add
MULT = mybir.AluOpType.mult


def build_kernel_nc(reps=1):
    nc = bacc.Bacc(
        "TRN2",
        target_bir_lowering=False,
        debug=False,
        enable_asserts=False,
        num_devices=NCORES,
    )
    enc = nc.dram_tensor("enc", [T, BC, H], BF16, kind="ExternalInput").ap()
    weT = nc.dram_tensor("WeT", [H, H], BF16, kind="ExternalInput").ap()
    whT = nc.dram_tensor("WhT", [H, H], BF16, kind="ExternalInput").ap()
    hidT = nc.dram_tensor("hidT", [H, BC], BF16, kind="ExternalInput").ap()
    bsum = nc.dram_tensor("bsum", [H], F32, kind="ExternalInput").ap()
    vrep = nc.dram_tensor("vrep", [128, H], BF16, kind="ExternalInput").ap()
    ones = nc.dram_tensor("ones", [128, 128], F32, kind="ExternalInput").ap()
    out = nc.dram_tensor("out", [BC, T], F32, kind="ExternalOutput").ap()
    scr_b = nc.dram_tensor("scr_b", [BC, H], F32, kind="ExternalOutput").ap()
    scr_s = nc.dram_tensor("scr_s", [BC, T], F32, kind="ExternalOutput").ap()

    with tile.TileContext(nc) as tc:
        _kernel_body(tc, enc, weT, whT, hidT, bsum, vrep, ones, out, scr_b,
                     scr_s, reps=reps)
    nc.compile()
    return nc


def _kernel_body(tc, enc, weT, whT, hidT, bsum, vrep, ones, out, scr_b,
                 scr_s, reps=1):
    nc = tc.nc
    with ExitStack() as ctx:
        singles = ctx.enter_context(tc.tile_pool(name="singles", bufs=1))

        # ---- persistent SBUF tensors -------------------------------------
        WeT = singles.tile([128, HC, H], BF16)   # WeT[h, hc, g] = W_e[g, 128*hc+h]
        v_rep = singles.tile([128, GH, TOK], BF16)   # v replicated to rows
        bias_rep = singles.tile([128, BC, GH, TOK], F32)  # hp+b_h+b_e bcast
        zeros_col = singles.tile([128, 1], F32)
        # scores2[p, b, c] = score of token t = c*128 + p of batch b
        scores2 = singles.tile([128, BC, CPB], F32)
        # batch b's score row lives on partition 32*b for the softmax
        scores = singles.tile([128, T], F32)
        probs = singles.tile([128, T], F32)
        negmax = singles.tile([128, 1], F32)
        sums = singles.tile([128, NEXPC], F32)
        stot = singles.tile([128, 1], F32)
        rsum = singles.tile([128, 1], F32)

        nc.gpsimd.memset(zeros_col[:], 0.0)
        nc.sync.dma_start(
            out=WeT[:], in_=weT.rearrange("(hc p) g -> p hc g", p=128)
        )
        nc.sync.dma_start(
            out=v_rep[:], in_=vrep.rearrange("p (gh c) -> p gh c", c=TOK)
        )

        # ---- stage 0: hp via proven [128,*] matmuls, then row-broadcast --
        with tc.tile_pool(name="stage0", bufs=1) as wload, tc.tile_pool(
            name="hpps", bufs=2, space="PSUM"
        ) as hp_pool, tc.tile_pool(name="bcps", bufs=2, space="PSUM") as bc_pool:
            WhT = wload.tile([128, HC, H], BF16)
            hidT_sb = wload.tile([128, HC, BC], BF16)
            bsum_sb = wload.tile([128, GC], F32)
            ones_sb = wload.tile([128, 128], F32)
            bias_all = wload.tile([128, GC, BC], F32)
            pad = wload.tile([128, BC, GH, TOK], F32)
            nc.sync.dma_start(
                out=WhT[:], in_=whT.rearrange("(hc p) g -> p hc g", p=128)
            )
            nc.sync.dma_start(
                out=hidT_sb[:], in_=hidT.rearrange("(hc p) b -> p hc b", p=128)
            )
            nc.sync.dma_start(
                out=bsum_sb[:], in_=bsum.rearrange("(c p) -> p c", p=128)
            )
            nc.sync.dma_start(out=ones_sb[:], in_=ones[:, :])
            nc.gpsimd.memset(pad[:], 0.0)

            # bias_all[g, gc, b] = hp^T + b_h + b_e  (the HW-proven path)
            for gc in range(GC):
                hp_ps = hp_pool.tile([128, BC], F32)
                for hc in range(HC):
                    nc.tensor.matmul(
                        hp_ps[:],
                        WhT[:, hc, ts(gc, 128)],
                        hidT_sb[:, hc, :],
                        start=(hc == 0),
                        stop=(hc == HC - 1),
                    )
                nc.vector.tensor_scalar(
                    out=bias_all[:, gc, :],
                    in0=hp_ps[:],
                    scalar1=bsum_sb[:, gc : gc + 1],
                    scalar2=None,
                    op0=mybir.AluOpType.add,
                )

            # Row-form: bounce bias_all through DRAM scratch (flat APs
            # allow the partition->free reorder), land the row on partition
            # 0 of the zeroed pad tile, then broadcast to all 128
            # partitions with an all-ones stationary matmul (sums over
            # partitions; only row 0 is nonzero).
            for b in range(BC):
                nc.sync.dma_start(
                    out=scr_b[b, :].rearrange("(gc p) -> p gc", p=128),
                    in_=bias_all[:, :, b],
                )
            for b in range(BC):
                for gh in range(GH):
                    nc.sync.dma_start(
                        out=pad[0:1, b, gh, :], in_=scr_b[b, ts(gh, TOK)]
                    )
            for b in range(BC):
                for gh in range(GH):
                    bc_ps = bc_pool.tile([128, TOK], F32)
                    nc.tensor.matmul(
                        bc_ps[:], ones_sb[:, :], pad[:, b, gh, :],
                        start=True, stop=True,
                    )
                    nc.vector.tensor_copy(bias_rep[:, b, gh, :], bc_ps[:])

        # ---- main loop: 8 supergroups of 1024 tokens ---------------------
        encT_pool = ctx.enter_context(tc.tile_pool(name="encT", bufs=3))
        xb_pool = ctx.enter_context(tc.tile_pool(name="xb", bufs=3))
        th_pool = ctx.enter_context(tc.tile_pool(name="th", bufs=3))
        ev_pool = ctx.enter_context(tc.tile_pool(name="ev", bufs=2))
        ep_pool = ctx.enter_context(tc.tile_pool(name="epps", bufs=4, space="PSUM"))

        n_total = reps * NSG

        def issue_load(sg):
            s = sg % NSG
            b = s // NSG_PER_B
            t0 = (s % NSG_PER_B) * SG
            encT = encT_pool.tile([128, HC, 2, TOK], BF16, tag="encT")
            for half in range(2):
                th_ = t0 + half * TOK
                nc.sync.dma_start_transpose(
                    out=encT[:, :, half, :], in_=enc[th_ : th_ + TOK, b, :]
                )
            return encT

        def softmax_head(b):
            # reorder scores2[:, b, :] into the [1, 2048] row at partition
            # 32*b via DRAM scratch (t = c*128 + p), then the negmax that
            # the interleaved exp chunks need
            r = slice(32 * b, 32 * b + 1)
            nc.sync.dma_start(
                out=scr_s[b, :].rearrange("(c p) -> p c", p=128),
                in_=scores2[:, b, :],
            )
            nc.sync.dma_start(out=scores[r, :], in_=scr_s[b, :])
            nc.vector.tensor_reduce(
                out=negmax[r], in_=scores[r, :], axis=AX.X,
                op=mybir.AluOpType.max, negate=True,
            )

        def softmax_chunk(b, c):
            r = slice(32 * b, 32 * b + 1)
            w = T // NEXPC
            nc.scalar.activation(
                out=probs[r, ts(c, w)], in_=scores[r, ts(c, w)], func=AF.Exp,
                bias=negmax[r], scale=1.0, accum_out=sums[r, c : c + 1],
            )

        def softmax_tail(b):
            r = slice(32 * b, 32 * b + 1)
            nc.vector.tensor_reduce(
                out=stot[r], in_=sums[r, :], axis=AX.X, op=ADD,
            )
            nc.vector.reciprocal(out=rsum[r], in_=stot[r])
            nc.vector.tensor_scalar_mul(probs[r, :], probs[r, :], rsum[r])
            nc.sync.dma_start(out=out[b : b + 1, :], in_=probs[r, :])

        encTs = [issue_load(0), issue_load(1)]
        sm_batch = None       # batch whose softmax chunks are pending

        for sg in range(n_total):
            s = sg % NSG
            b = s // NSG_PER_B
            c0 = (s % NSG_PER_B) * TB

            if sg + 2 < n_total:
                encTs.append(issue_load(sg + 2))
            encT_cur = encTs[sg]

            for tb in range(TB):
                ev = ev_pool.tile([128, GH, TOK], BF16)
                for gh in range(GH):
                    ep_ps = ep_pool.tile([128, TOK], F32)
                    for hc in range(HC):
                        nc.tensor.matmul(
                            ep_ps[:],
                            encT_cur[:, hc, tb // 4, ts(tb % 4, 128)],
                            WeT[:, hc, ts(gh, TOK)],
                            start=(hc == 0),
                            stop=(hc == HC - 1),
                        )
                    xb = xb_pool.tile([128, TOK], F32)
                    nc.vector.tensor_add(
                        xb[:], ep_ps[:], bias_rep[:, b, gh, :]
                    )
                    th = th_pool.tile([128, TOK], BF16)
                    nc.scalar.activation(
                        out=th[:], in_=xb[:], func=AF.Tanh,
                        bias=zeros_col[:], scale=1.0,
                    )
                    # en * v into the half of the per-tblock product tile
                    nc.vector.tensor_mul(
                        ev[:, gh, :], th[:], v_rep[:, gh, :]
                    )
                    # one softmax-exp chunk of the previous batch per 4 chains
                    ci = tb * GH + gh
                    if sm_batch is not None and ci % 4 == 3:
                        softmax_chunk(sm_batch, ci // 4)
                        if ci == 15:
                            softmax_tail(sm_batch)
                            sm_batch = None
                # one free-axis reduce over both halves -> score column
                nc.vector.tensor_reduce(
                    out=scores2[:, b, c0 + tb : c0 + tb + 1],
                    in_=ev[:].rearrange("p a c -> p (a c)"), axis=AX.X, op=ADD,
                )

            if s % NSG_PER_B == NSG_PER_B - 1:
                softmax_head(b)
                sm_batch = b
            encTs[sg] = None  # release reference

        if sm_batch is not None:
            for c in range(NEXPC):
                softmax_chunk(sm_batch, c)
            softmax_tail(sm_batch)


_NC_CACHE = None


def _get_nc():
    global _NC_CACHE
    if _NC_CACHE is None:
        _NC_CACHE = build_kernel_nc()
    return _NC_CACHE


def make_in_maps(hidden, encoder_outputs, W_h, b_h, W_e, b_e, v):
    hidden = np.asarray(hidden, dtype=np.float32)
    enc = np.asarray(encoder_outputs, dtype=np.float32)
    W_h = np.asarray(W_h, dtype=np.float32)
    W_e = np.asarray(W_e, dtype=np.float32)
    b_h = np.asarray(b_h, dtype=np.float32)
    b_e = np.asarray(b_e, dtype=np.float32)
    v = np.asarray(v, dtype=np.float32)

    enc_bf = enc.astype(ml_dtypes.bfloat16)
    weT = np.ascontiguousarray(W_e.T).astype(ml_dtypes.bfloat16)
    whT = np.ascontiguousarray(W_h.T).astype(ml_dtypes.bfloat16)
    bsum = np.ascontiguousarray(b_h + b_e)
    vrep = np.ascontiguousarray(
        np.broadcast_to(v[None, :], (128, H))
    ).astype(ml_dtypes.bfloat16)
    ones = np.ones((128, 128), dtype=np.float32)
    hid0 = hidden.reshape(B, H)

    in_maps = []
    for c in range(NCORES):
        hidT = np.ascontiguousarray(
            hid0[c * BC : (c + 1) * BC, :].T
        ).astype(ml_dtypes.bfloat16)
        in_maps.append(
            {
                "enc": np.ascontiguousarray(enc_bf[:, c * BC : (c + 1) * BC, :]),
                "WeT": weT,
                "WhT": whT,
                "hidT": hidT,
                "bsum": bsum,
                "vrep": vrep,
                "ones": ones,
            }
        )
    return in_maps


def kernel(hidden, encoder_outputs, W_h, b_h, W_e, b_e, v):
    nc = _get_nc()
    in_maps = make_in_maps(hidden, encoder_outputs, W_h, b_h, W_e, b_e, v)
    res = run_bass_kernel_spmd(nc, in_maps, list(range(NCORES)))
    full = np.concatenate([res.results[c]["out"] for c in range(NCORES)], axis=0)
    return full[:, None, :].astype(np.float32)
